# revision 4
# baseline (speedup 1.0000x reference)
"""Trainium2 kernel for GraphConvolution_multi_avg (AAGNN).

Computes out = relu((adj @ (x @ W)) * degree_norm / num_avg + b) for
N=16384, F=128, H=64 on 8 NeuronCores.

Sharding: rows of adj / degree_norm / output are split across the 8
cores (2048 rows each); x, W, b are replicated. No collectives — each
core produces its own output rows.

Per-core device kernel (all heavy math on TensorE, bf16 inputs with
fp32 PSUM accumulation):
  - support = (x @ (W/num_avg)) computed from a replicated x^T
    ([128, 16384]) so each 128-node tile of support lands with nodes on
    partitions, ready to serve as the stationary matmul operand.
  - aggT[h, r] = sum_k support[k, h] * adjT[k, r] accumulated over 128
    k-tiles into 4 PSUM banks ([64, 4, 512]). The moving operand is the
    host-pre-transposed adjacency shard adjT [16384, 2048] (bf16),
    streamed tile-by-tile ([128, 2048] = 512 KiB per DMA).
  - epilogue: aggT * degree_norm (broadcast along partitions) then
    relu(. + b) on ScalarE, DMA out as outT [64, 2048]; the host
    transposes back.
"""

import numpy as np
import ml_dtypes

import concourse.bass as bass  # noqa: F401  (engine types come via nc)
import concourse.mybir as mybir
import concourse.tile as tile
from concourse import bacc
from concourse.bass_utils import run_bass_kernel_spmd

N, F, H = 16384, 128, 64
NCORES = 8
P = 128
R = N // NCORES          # 2048 local rows per core
KT = N // P              # 128 contraction (node) tiles
RBS = 512                # r-block size = one PSUM bank of fp32
RB = R // RBS            # 4 r-blocks
ADJ_BUFS = 12            # adjT stream ring depth (12 * 4 KiB/partition)

_BF16 = ml_dtypes.bfloat16
_NC_CACHE: dict = {}


def _build(inv_avg: float):
    nc = bacc.Bacc("TRN2", target_bir_lowering=False, debug=False)
    bf16 = mybir.dt.bfloat16
    f32 = mybir.dt.float32

    adjt = nc.dram_tensor("adjt", [KT, P, R], bf16, kind="ExternalInput")
    xt = nc.dram_tensor("xt", [F, N], bf16, kind="ExternalInput")
    w = nc.dram_tensor("w", [F, H], bf16, kind="ExternalInput")
    # degree_norm shard pre-replicated across the H partitions on host.
    dn = nc.dram_tensor("dn", [H, R], f32, kind="ExternalInput")
    bvec = nc.dram_tensor("bvec", [H], f32, kind="ExternalInput")
    out = nc.dram_tensor("out", [H, R], f32, kind="ExternalOutput")

    with tile.TileContext(nc) as tc:
        with (
            tc.tile_pool(name="const", bufs=1) as const,
            tc.tile_pool(name="adj", bufs=ADJ_BUFS) as adjp,
            tc.tile_pool(name="psA", bufs=1, space="PSUM") as psA,
            tc.tile_pool(name="psS", bufs=2, space="PSUM") as psS,
            tc.tile_pool(name="ep", bufs=3) as ep,
        ):
            xt_sb = const.tile([F, N], bf16, name="xt_sb")
            nc.sync.dma_start(xt_sb[:], xt.ap())
            w_sb = const.tile([F, H], bf16, name="w_sb")
            nc.sync.dma_start(w_sb[:], w.ap())
            # Fold 1/num_avg into W so the epilogue only needs the raw
            # degree_norm multiply.
            w_sc = const.tile([F, H], bf16, name="w_sc")
            nc.vector.tensor_scalar_mul(w_sc[:], w_sb[:], inv_avg)

            # degree_norm broadcast to all H partitions: [64, 2048] fp32.
            dnb = const.tile([H, R], f32, name="dnb")
            nc.sync.dma_start(dnb[:], dn.ap())
            b_sb = const.tile([H, 1], f32, name="b_sb")
            nc.sync.dma_start(b_sb[:], bvec.ap().unsqueeze(1))

            # support[p, kt, h] = (x @ W/num_avg)[kt*128 + p, h], bf16.
            support = const.tile([P, KT, H], bf16, name="support")
            for nt in range(KT):
                ps = psS.tile([P, H], f32, name="ps_supp")
                nc.tensor.matmul(
                    ps[:],
                    lhsT=xt_sb[:, nt * P:(nt + 1) * P],
                    rhs=w_sc[:],
                    start=True,
                    stop=True,
                )
                nc.vector.tensor_copy(support[:, nt, :], ps[:])

            # aggT accumulator: [64, 4, 512] fp32 = 4 PSUM banks.
            aggps = psA.tile([H, RB, RBS], f32, name="aggps")
            for kt in range(KT):
                at = adjp.tile([P, R], bf16, name="at")
                # Alternate HWDGE rings (SP / ACT) for DMA parallelism.
                eng = nc.sync if (kt % 2 == 0) else nc.scalar
                eng.dma_start(at[:], adjt.ap()[kt])
                for rb in range(RB):
                    nc.tensor.matmul(
                        aggps[:, rb, :],
                        lhsT=support[:, kt, :],
                        rhs=at[:, rb * RBS:(rb + 1) * RBS],
                        start=(kt == 0),
                        stop=(kt == KT - 1),
                    )

            for rb in range(RB):
                h_sb = ep.tile([H, RBS], f32, name="h_sb")
                nc.vector.tensor_mul(
                    out=h_sb[:],
                    in0=aggps[:, rb, :],
                    in1=dnb[:, rb * RBS:(rb + 1) * RBS],
                )
                o_sb = ep.tile([H, RBS], f32, name="o_sb")
                nc.scalar.activation(
                    o_sb[:],
                    h_sb[:],
                    mybir.ActivationFunctionType.Relu,
                    bias=b_sb[:],
                    scale=1.0,
                )
                nc.sync.dma_start(out.ap()[:, rb * RBS:(rb + 1) * RBS], o_sb[:])

    nc.compile()
    return nc


def _get_nc(inv_avg: float):
    key = round(float(inv_avg), 12)
    if key not in _NC_CACHE:
        _NC_CACHE[key] = _build(float(inv_avg))
    return _NC_CACHE[key]


def _make_in_maps(x, adj_matrix, degree_norm, W, b):
    x = np.asarray(x, dtype=np.float32).reshape(N, F)
    adj = np.asarray(adj_matrix, dtype=np.float32).reshape(N, N)
    dn = np.asarray(degree_norm, dtype=np.float32).reshape(N)
    Wm = np.asarray(W, dtype=np.float32).reshape(F, H)
    bv = np.asarray(b, dtype=np.float32).reshape(H)

    xt = x.T.astype(_BF16, order="C")          # [128, 16384]
    wb = Wm.astype(_BF16, order="C")           # [128, 64]
    in_maps = []
    for c in range(NCORES):
        rows = slice(c * R, (c + 1) * R)
        adjt_c = adj[rows, :].T.astype(_BF16, order="C")   # [16384, 2048]
        in_maps.append({
            "adjt": adjt_c.reshape(KT, P, R),
            "xt": xt,
            "w": wb,
            "dn": np.ascontiguousarray(np.broadcast_to(dn[rows][None, :], (H, R))),
            "bvec": bv,
        })
    return in_maps


def _run(inputs: dict, trace: bool = False, **run_kwargs):
    num_avg = inputs["num_avg"]
    inv_avg = 1.0 / float(num_avg)
    nc = _get_nc(inv_avg)
    in_maps = _make_in_maps(
        inputs["x"], inputs["adj_matrix"], inputs["degree_norm"],
        inputs["W"], inputs["b"],
    )
    res = run_bass_kernel_spmd(
        nc, in_maps, core_ids=list(range(NCORES)), trace=trace, **run_kwargs
    )
    outf = np.empty((N, H), dtype=np.float32)
    for c in range(NCORES):
        outf[c * R:(c + 1) * R, :] = np.asarray(res.results[c]["out"]).T
    return outf, res


def kernel(**inputs) -> np.ndarray:
    return _run(inputs, trace=False)[0]


# revision 6
# speedup vs baseline: 1.1250x; 1.1250x over previous
"""Trainium2 kernel for GraphConvolution_multi_avg (AAGNN).

Computes out = relu((adj @ (x @ W)) * degree_norm / num_avg + b) for
N=16384, F=128, H=64 on 8 NeuronCores.

Sharding: rows of adj / degree_norm / output are split across the 8
cores (2048 rows each); x, W, b are replicated. No collectives — each
core produces its own output rows.

Per-core device kernel (all heavy math on TensorE, bf16 inputs with
fp32 PSUM accumulation):
  - support = (x @ (W/num_avg)) computed from a replicated x^T
    ([128, 16384]) so each 128-node tile of support lands with nodes on
    partitions, ready to serve as the stationary matmul operand.
  - aggT[h, r] = sum_k support[k, h] * adjT[k, r] accumulated over 128
    k-tiles into 4 PSUM banks ([64, 4, 512]). The moving operand is the
    host-pre-transposed adjacency shard adjT [16384, 2048] (bf16),
    streamed tile-by-tile ([128, 2048] = 512 KiB per DMA).
  - epilogue: aggT * degree_norm (broadcast along partitions) then
    relu(. + b) on ScalarE, DMA out as outT [64, 2048]; the host
    transposes back.
"""

import numpy as np
import ml_dtypes

import concourse.bass as bass  # noqa: F401  (engine types come via nc)
import concourse.mybir as mybir
import concourse.tile as tile
from concourse import bacc
from concourse.bass_utils import run_bass_kernel_spmd

N, F, H = 16384, 128, 64
NCORES = 8
P = 128
R = N // NCORES          # 2048 local rows per core
KT = N // P              # 128 contraction (node) tiles
RBS = 512                # r-block size = one PSUM bank of fp32
RB = R // RBS            # 4 r-blocks
ADJ_BUFS = 12            # adjT stream ring depth (12 * 4 KiB/partition)

_BF16 = ml_dtypes.bfloat16
_NC_CACHE: dict = {}


def _build(inv_avg: float):
    nc = bacc.Bacc("TRN2", target_bir_lowering=False, debug=False)
    bf16 = mybir.dt.bfloat16
    f32 = mybir.dt.float32

    adjt = nc.dram_tensor("adjt", [KT, P, R], bf16, kind="ExternalInput")
    xt = nc.dram_tensor("xt", [F, N], bf16, kind="ExternalInput")
    w = nc.dram_tensor("w", [F, H], bf16, kind="ExternalInput")
    # degree_norm shard pre-replicated across the H partitions on host.
    dn = nc.dram_tensor("dn", [H, R], f32, kind="ExternalInput")
    bvec = nc.dram_tensor("bvec", [H], f32, kind="ExternalInput")
    out = nc.dram_tensor("out", [H, R], f32, kind="ExternalOutput")

    with tile.TileContext(nc) as tc:
        with (
            tc.tile_pool(name="const", bufs=1) as const,
            tc.tile_pool(name="adj", bufs=ADJ_BUFS) as adjp,
            tc.tile_pool(name="psA", bufs=1, space="PSUM") as psA,
            tc.tile_pool(name="psS", bufs=3, space="PSUM") as psS,
            tc.tile_pool(name="ep", bufs=3) as ep,
        ):
            # xt load split across both HWDGE rings so the first chunk (all
            # the support compute needs to start) lands in ~1 us.
            xt_sb = const.tile([F, N], bf16, name="xt_sb")
            XTC = 8
            xc = N // XTC
            for i in range(XTC):
                eng = nc.sync if i % 2 == 0 else nc.scalar
                eng.dma_start(
                    xt_sb[:, i * xc:(i + 1) * xc],
                    xt.ap()[:, i * xc:(i + 1) * xc],
                )
            # Small constants go via SWDGE (gpsimd) to keep the HW rings
            # free for the adjacency stream.
            w_sb = const.tile([F, H], bf16, name="w_sb")
            nc.gpsimd.dma_start(w_sb[:], w.ap())
            # Fold 1/num_avg into W so the epilogue only needs the raw
            # degree_norm multiply.
            w_sc = const.tile([F, H], bf16, name="w_sc")
            nc.vector.tensor_scalar_mul(w_sc[:], w_sb[:], inv_avg)

            # degree_norm broadcast to all H partitions: [64, 2048] fp32.
            dnb = const.tile([H, R], f32, name="dnb")
            nc.gpsimd.dma_start(dnb[:], dn.ap())
            b_sb = const.tile([H, 1], f32, name="b_sb")
            nc.gpsimd.dma_start(b_sb[:], bvec.ap().unsqueeze(1))

            # support[p, kt, h] = (x @ W/num_avg)[kt*128 + p, h], bf16.
            # Produced tile-by-tile, interleaved with the big matmuls so the
            # PE never sits on a long support-only prolog.
            support = const.tile([P, KT, H], bf16, name="support")

            def emit_support(nt):
                ps = psS.tile([P, H], f32, name="ps_supp")
                nc.tensor.matmul(
                    ps[:],
                    lhsT=xt_sb[:, nt * P:(nt + 1) * P],
                    rhs=w_sc[:],
                    start=True,
                    stop=True,
                )
                nc.vector.tensor_copy(support[:, nt, :], ps[:])

            # aggT accumulator: [64, 4, 512] fp32 = 4 PSUM banks.
            aggps = psA.tile([H, RB, RBS], f32, name="aggps")
            emit_support(0)
            emit_support(1)
            for kt in range(KT):
                at = adjp.tile([P, R], bf16, name="at")
                # Alternate HWDGE rings (SP / ACT) for DMA parallelism.
                eng = nc.sync if (kt % 2 == 0) else nc.scalar
                eng.dma_start(at[:], adjt.ap()[kt])
                if kt + 2 < KT:
                    emit_support(kt + 2)
                for rb in range(RB):
                    nc.tensor.matmul(
                        aggps[:, rb, :],
                        lhsT=support[:, kt, :],
                        rhs=at[:, rb * RBS:(rb + 1) * RBS],
                        start=(kt == 0),
                        stop=(kt == KT - 1),
                    )

            for rb in range(RB):
                h_sb = ep.tile([H, RBS], f32, name="h_sb")
                nc.vector.tensor_mul(
                    out=h_sb[:],
                    in0=aggps[:, rb, :],
                    in1=dnb[:, rb * RBS:(rb + 1) * RBS],
                )
                o_sb = ep.tile([H, RBS], f32, name="o_sb")
                nc.scalar.activation(
                    o_sb[:],
                    h_sb[:],
                    mybir.ActivationFunctionType.Relu,
                    bias=b_sb[:],
                    scale=1.0,
                )
                nc.sync.dma_start(out.ap()[:, rb * RBS:(rb + 1) * RBS], o_sb[:])

    nc.compile()
    return nc


def _get_nc(inv_avg: float):
    key = round(float(inv_avg), 12)
    if key not in _NC_CACHE:
        _NC_CACHE[key] = _build(float(inv_avg))
    return _NC_CACHE[key]


def _make_in_maps(x, adj_matrix, degree_norm, W, b):
    x = np.asarray(x, dtype=np.float32).reshape(N, F)
    adj = np.asarray(adj_matrix, dtype=np.float32).reshape(N, N)
    dn = np.asarray(degree_norm, dtype=np.float32).reshape(N)
    Wm = np.asarray(W, dtype=np.float32).reshape(F, H)
    bv = np.asarray(b, dtype=np.float32).reshape(H)

    xt = x.T.astype(_BF16, order="C")          # [128, 16384]
    wb = Wm.astype(_BF16, order="C")           # [128, 64]
    in_maps = []
    for c in range(NCORES):
        rows = slice(c * R, (c + 1) * R)
        adjt_c = adj[rows, :].T.astype(_BF16, order="C")   # [16384, 2048]
        in_maps.append({
            "adjt": adjt_c.reshape(KT, P, R),
            "xt": xt,
            "w": wb,
            "dn": np.ascontiguousarray(np.broadcast_to(dn[rows][None, :], (H, R))),
            "bvec": bv,
        })
    return in_maps


def _run(inputs: dict, trace: bool = False, **run_kwargs):
    num_avg = inputs["num_avg"]
    inv_avg = 1.0 / float(num_avg)
    nc = _get_nc(inv_avg)
    in_maps = _make_in_maps(
        inputs["x"], inputs["adj_matrix"], inputs["degree_norm"],
        inputs["W"], inputs["b"],
    )
    res = run_bass_kernel_spmd(
        nc, in_maps, core_ids=list(range(NCORES)), trace=trace, **run_kwargs
    )
    outf = np.empty((N, H), dtype=np.float32)
    for c in range(NCORES):
        outf[c * R:(c + 1) * R, :] = np.asarray(res.results[c]["out"]).T
    return outf, res


def kernel(**inputs) -> np.ndarray:
    return _run(inputs, trace=False)[0]


# revision 8
# speedup vs baseline: 1.1567x; 1.0282x over previous
"""Trainium2 kernel for GraphConvolution_multi_avg (AAGNN).

Computes out = relu((adj @ (x @ W)) * degree_norm / num_avg + b) for
N=16384, F=128, H=64 on 8 NeuronCores.

Sharding: rows of adj / degree_norm / output are split across the 8
cores (2048 rows each); x, W, b are replicated. No collectives — each
core produces its own output rows.

Per-core device kernel (all heavy math on TensorE, bf16 inputs with
fp32 PSUM accumulation):
  - support = (x @ (W/num_avg)) computed from a replicated x^T
    ([128, 16384]) so each 128-node tile of support lands with nodes on
    partitions, ready to serve as the stationary matmul operand.
  - aggT[h, r] = sum_k support[k, h] * adjT[k, r] accumulated over 128
    k-tiles into 4 PSUM banks ([64, 4, 512]). The moving operand is the
    host-pre-transposed adjacency shard adjT [16384, 2048] (bf16),
    streamed tile-by-tile ([128, 2048] = 512 KiB per DMA).
  - epilogue: aggT * degree_norm (broadcast along partitions) then
    relu(. + b) on ScalarE, DMA out as outT [64, 2048]; the host
    transposes back.
"""

import numpy as np
import ml_dtypes

import concourse.bass as bass  # noqa: F401  (engine types come via nc)
import concourse.mybir as mybir
import concourse.tile as tile
from concourse import bacc
from concourse.bass_utils import run_bass_kernel_spmd

N, F, H = 16384, 128, 64
NCORES = 8
P = 128
R = N // NCORES          # 2048 local rows per core
KT = N // P              # 128 contraction (node) tiles
RBS = 512                # r-block size = one PSUM bank of fp32
RB = R // RBS            # 4 r-blocks
ADJ_BUFS = 24            # adjT stream ring depth (24 * 4 KiB/partition)

_BF16 = ml_dtypes.bfloat16
_NC_CACHE: dict = {}


def _build(inv_avg: float):
    nc = bacc.Bacc("TRN2", target_bir_lowering=False, debug=False)
    bf16 = mybir.dt.bfloat16
    f32 = mybir.dt.float32

    adjt = nc.dram_tensor("adjt", [KT, P, R], bf16, kind="ExternalInput")
    xt = nc.dram_tensor("xt", [F, N], bf16, kind="ExternalInput")
    w = nc.dram_tensor("w", [F, H], bf16, kind="ExternalInput")
    # degree_norm shard pre-replicated across the H partitions on host.
    dn = nc.dram_tensor("dn", [H, R], f32, kind="ExternalInput")
    bvec = nc.dram_tensor("bvec", [H], f32, kind="ExternalInput")
    out = nc.dram_tensor("out", [H, R], f32, kind="ExternalOutput")

    with tile.TileContext(nc) as tc:
        with (
            tc.tile_pool(name="const", bufs=1) as const,
            tc.tile_pool(name="adj", bufs=ADJ_BUFS) as adjp,
            tc.tile_pool(name="psA", bufs=1, space="PSUM") as psA,
            tc.tile_pool(name="psS", bufs=3, space="PSUM") as psS,
            tc.tile_pool(name="ep", bufs=3) as ep,
        ):
            # xt load split across both HWDGE rings so the first chunk (all
            # the support compute needs to start) lands in ~1 us.
            xt_sb = const.tile([F, N], bf16, name="xt_sb")
            XTC = 8
            xc = N // XTC
            for i in range(XTC):
                eng = nc.sync if i % 2 == 0 else nc.scalar
                eng.dma_start(
                    xt_sb[:, i * xc:(i + 1) * xc],
                    xt.ap()[:, i * xc:(i + 1) * xc],
                )
            # Small constants go via SWDGE (gpsimd) to keep the HW rings
            # free for the adjacency stream.
            w_sb = const.tile([F, H], bf16, name="w_sb")
            nc.gpsimd.dma_start(w_sb[:], w.ap())
            # Fold 1/num_avg into W so the epilogue only needs the raw
            # degree_norm multiply.
            w_sc = const.tile([F, H], bf16, name="w_sc")
            nc.vector.tensor_scalar_mul(w_sc[:], w_sb[:], inv_avg)

            # degree_norm broadcast to all H partitions: [64, 2048] fp32.
            dnb = const.tile([H, R], f32, name="dnb")
            nc.gpsimd.dma_start(dnb[:], dn.ap())
            b_sb = const.tile([H, 1], f32, name="b_sb")
            nc.gpsimd.dma_start(b_sb[:], bvec.ap().unsqueeze(1))

            # support[p, kt, h] = (x @ W/num_avg)[kt*128 + p, h], bf16.
            # Separate prolog phase (~20 us, LDWEIGHTS-bound): 8 node-tiles
            # share one PSUM bank so the fp32->bf16 cast is one batched DVE
            # copy per 8 matmuls. The adjacency DMA streams into the deep
            # ring during this phase, so DMA never idles.
            support = const.tile([P, KT, H], bf16, name="support")
            SUPP_BATCH = RBS // H  # 8 node-tiles per PSUM bank
            for g in range(KT // SUPP_BATCH):
                ps = psS.tile([P, RBS], f32, name="ps_supp")
                for j in range(SUPP_BATCH):
                    nt = g * SUPP_BATCH + j
                    nc.tensor.matmul(
                        ps[:, j * H:(j + 1) * H],
                        lhsT=xt_sb[:, nt * P:(nt + 1) * P],
                        rhs=w_sc[:],
                        start=True,
                        stop=True,
                    )
                nc.vector.tensor_copy(
                    support[:, g * SUPP_BATCH:(g + 1) * SUPP_BATCH, :], ps[:]
                )

            # aggT accumulator: [64, 4, 512] fp32 = 4 PSUM banks. Main loop
            # is pure big-matmul streaming: no weight-set ping-pong bubbles.
            aggps = psA.tile([H, RB, RBS], f32, name="aggps")
            for kt in range(KT):
                at = adjp.tile([P, R], bf16, name="at")
                # Alternate HWDGE rings (SP / ACT) for DMA parallelism.
                eng = nc.sync if (kt % 2 == 0) else nc.scalar
                eng.dma_start(at[:], adjt.ap()[kt])
                for rb in range(RB):
                    nc.tensor.matmul(
                        aggps[:, rb, :],
                        lhsT=support[:, kt, :],
                        rhs=at[:, rb * RBS:(rb + 1) * RBS],
                        start=(kt == 0),
                        stop=(kt == KT - 1),
                    )

            for rb in range(RB):
                h_sb = ep.tile([H, RBS], f32, name="h_sb")
                nc.vector.tensor_mul(
                    out=h_sb[:],
                    in0=aggps[:, rb, :],
                    in1=dnb[:, rb * RBS:(rb + 1) * RBS],
                )
                o_sb = ep.tile([H, RBS], f32, name="o_sb")
                nc.scalar.activation(
                    o_sb[:],
                    h_sb[:],
                    mybir.ActivationFunctionType.Relu,
                    bias=b_sb[:],
                    scale=1.0,
                )
                nc.sync.dma_start(out.ap()[:, rb * RBS:(rb + 1) * RBS], o_sb[:])

    nc.compile()
    return nc


def _get_nc(inv_avg: float):
    key = round(float(inv_avg), 12)
    if key not in _NC_CACHE:
        _NC_CACHE[key] = _build(float(inv_avg))
    return _NC_CACHE[key]


def _make_in_maps(x, adj_matrix, degree_norm, W, b):
    x = np.asarray(x, dtype=np.float32).reshape(N, F)
    adj = np.asarray(adj_matrix, dtype=np.float32).reshape(N, N)
    dn = np.asarray(degree_norm, dtype=np.float32).reshape(N)
    Wm = np.asarray(W, dtype=np.float32).reshape(F, H)
    bv = np.asarray(b, dtype=np.float32).reshape(H)

    xt = x.T.astype(_BF16, order="C")          # [128, 16384]
    wb = Wm.astype(_BF16, order="C")           # [128, 64]
    in_maps = []
    for c in range(NCORES):
        rows = slice(c * R, (c + 1) * R)
        adjt_c = adj[rows, :].T.astype(_BF16, order="C")   # [16384, 2048]
        in_maps.append({
            "adjt": adjt_c.reshape(KT, P, R),
            "xt": xt,
            "w": wb,
            "dn": np.ascontiguousarray(np.broadcast_to(dn[rows][None, :], (H, R))),
            "bvec": bv,
        })
    return in_maps


def _run(inputs: dict, trace: bool = False, **run_kwargs):
    num_avg = inputs["num_avg"]
    inv_avg = 1.0 / float(num_avg)
    nc = _get_nc(inv_avg)
    in_maps = _make_in_maps(
        inputs["x"], inputs["adj_matrix"], inputs["degree_norm"],
        inputs["W"], inputs["b"],
    )
    res = run_bass_kernel_spmd(
        nc, in_maps, core_ids=list(range(NCORES)), trace=trace, **run_kwargs
    )
    outf = np.empty((N, H), dtype=np.float32)
    for c in range(NCORES):
        outf[c * R:(c + 1) * R, :] = np.asarray(res.results[c]["out"]).T
    return outf, res


def kernel(**inputs) -> np.ndarray:
    return _run(inputs, trace=False)[0]
